# revision 31
# baseline (speedup 1.0000x reference)
"""AdditiveAttention on Trainium2 (Bass/Tile), 8 cores, valid_lens-aware resharding.

Reference per batch b:
  q = queries @ Wq; k = keys @ Wk
  scores[i,j] = wv . tanh(q[i] + k[j]); masked softmax over j; out = attn @ values

Masked columns (j >= valid_len) contribute exactly zero, so only
ceil(valid_len/256) 256-wide j-slots per batch need computing. Work units of
(batch, 64 query rows) are packed two per core (unit A -> u=0, unit B -> u=1)
with S_A slots for A and S_B for B (largest-with-smallest pairing; short
units get zero-key, -1e6-masked pad slots). One SPMD program: all per-core
variation lives in input DATA (slot-gathered keys/values/mask, stacked
queries); instructions are identical on every core.

Row permutation i_phys(u, l) = 32*(l%4) + 16*u + l//4 puts consecutive l on
disjoint PE column groups (4-way tile_position concurrency); the one-hot
column of the wv stationary (shared constant data) routes each matmul's
result to that row.

Engine split: PE projects kT/qT; DVE builds preact via tensor_scalar_add
(per-partition scalar q column, FD = S_A*256 per call via slot-interleaved
layout); ACT does tanh in 2*(2+S_B... ) long N=16K calls (the bottleneck)
plus the final exp (no max-subtraction: |scores| <= sum|wv| ~ 13); PE
accumulates scores via one-hot wv matmuls and computes attn @ values.
"""

import numpy as np
import ml_dtypes
from contextlib import ExitStack

from concourse import bacc, tile
import concourse.bass as bass
import concourse.mybir as mybir
from concourse.bass_utils import run_bass_kernel_spmd

F32 = mybir.dt.float32
F32R = mybir.dt.float32r
BF16 = mybir.dt.bfloat16
AF = mybir.ActivationFunctionType
ts = bass.ts

B, Lq, Lk, D, H = 8, 128, 512, 256, 256
NCORES = 8
JB = 256           # j-slot width

_CACHE = {}


def i_phys(u, l):
    return 32 * (l % 4) + 16 * u + l // 4


def build_program(SA, SB):
    nc = bacc.Bacc(
        "TRN2", target_bir_lowering=False, debug=False, enable_asserts=False
    )

    S = SA + SB
    W = S * JB          # gathered scores width
    WA, WB = SA * JB, SB * JB
    qsT_d = nc.dram_tensor("qsT", [128, D], BF16, kind="ExternalInput")
    keysT_d = nc.dram_tensor("keysT", [128, 2 * W], BF16, kind="ExternalInput")
    Wq_d = nc.dram_tensor("Wq", [128, 2 * H], BF16, kind="ExternalInput")
    Wk_d = nc.dram_tensor("Wk", [128, 2 * H], BF16, kind="ExternalInput")
    mask_d = nc.dram_tensor("mask", [128, W], BF16, kind="ExternalInput")
    identb_d = nc.dram_tensor("identb", [128, 128], BF16, kind="ExternalInput")
    ident_d = nc.dram_tensor("ident", [128, 128], F32, kind="ExternalInput")
    # wv32[(u*2+t)*128 + k, l*32 + r] = wv[t*128+k] iff r == 16*u + l//4
    wv32_d = nc.dram_tensor("wv32", [128, 4 * 64 * 32], BF16, kind="ExternalInput")
    out_d = nc.dram_tensor("out", [Lq, H], F32, kind="ExternalOutput")

    NJ6 = W // 128      # 128-row j-blocks of the gathered axis (for attn@values)
    values_d = nc.dram_tensor("values", [128, NJ6 * H], BF16, kind="ExternalInput")

    with tile.TileContext(nc) as tc, ExitStack() as ctx:
        const = ctx.enter_context(tc.tile_pool(name="const", bufs=1))
        inp = ctx.enter_context(tc.tile_pool(name="inp", bufs=1))
        proj = ctx.enter_context(tc.tile_pool(name="proj", bufs=1))
        prep = ctx.enter_context(tc.tile_pool(name="prep", bufs=2))
        featp = ctx.enter_context(tc.tile_pool(name="featp", bufs=2))
        sm = ctx.enter_context(tc.tile_pool(name="sm", bufs=1))
        ps_big = ctx.enter_context(tc.tile_pool(name="ps_big", bufs=1, space="PSUM"))
        ps_sc = ctx.enter_context(tc.tile_pool(name="ps_sc", bufs=1, space="PSUM"))
        ps_sm = ctx.enter_context(tc.tile_pool(name="ps_sm", bufs=1, space="PSUM"))

        # ---- ACT spline table warmup (tanh/exp share a set); no DMA dep ----
        warm_in = sm.tile([1, 2], F32)
        nc.vector.memset(warm_in[:], 0.0)
        warm_sb = sm.tile([1, 2], F32)
        nc.scalar.activation(warm_sb[0:1, 0:1], warm_in[0:1, 0:1], AF.Tanh)
        nc.scalar.activation(warm_sb[0:1, 1:2], warm_in[0:1, 0:1], AF.Exp)

        # ---- input loads; k-projection path first (it gates the pipeline) ----
        HW2 = W // 2
        keysT_sb = inp.tile([128, 2 * W], BF16)  # [d', dt*W + j]
        nc.sync.dma_start(keysT_sb[:, 0:W], keysT_d[:, 0:W])
        nc.scalar.dma_start(keysT_sb[:, W : 2 * W], keysT_d[:, W : 2 * W])
        qsT_sb = inp.tile([128, D], BF16)  # [d', dt*128 + (u*64+l)]
        nc.scalar.dma_start(qsT_sb[:], qsT_d[:])
        Wq_sb = inp.tile([128, 2 * H], BF16)
        nc.scalar.dma_start(Wq_sb[:], Wq_d[:])
        Wk_sb = inp.tile([128, 2 * H], BF16)  # [d', dt*256 + h]
        nc.gpsimd.dma_start(Wk_sb[:], Wk_d[:])
        mask_sb = const.tile([128, W], BF16)
        nc.gpsimd.dma_start(mask_sb[:], mask_d[:])
        identb_sb = const.tile([128, 128], BF16)
        nc.sync.dma_start(identb_sb[:], identb_d[:])
        ident_sb = const.tile([128, 128], F32)
        nc.sync.dma_start(ident_sb[:], ident_d[:])
        w32_sb = const.tile([128, 4 * 64 * 32], BF16)  # [(u*2+t) blocks]
        nc.gpsimd.dma_start(w32_sb[:], wv32_d[:])
        values_r = inp.tile([128, NJ6 * H], BF16)  # [j', jb*256 + v]
        nc.gpsimd.dma_start(values_r[:], values_d[:])

        # ---- projections (q first: its inputs land earlier) ----
        # kT_ps laid out [t*1024 + jh*512 .. +W/2] so no matmul output
        # crosses a PSUM bank boundary
        kT_ps = ps_big.tile([128, 2048], F32, tag="big")
        for t in range(2):
            for jh in range(2):
                for dt in range(2):
                    nc.tensor.matmul(
                        kT_ps[:, t * 1024 + jh * 512 : t * 1024 + jh * 512 + HW2],
                        Wk_sb[:, dt * H + t * 128 : dt * H + t * 128 + 128],
                        keysT_sb[:, dt * W + jh * HW2 : dt * W + jh * HW2 + HW2],
                        start=(dt == 0),
                        stop=(dt == 1),
                    )
        kT_sb = proj.tile([128, 2 * W], BF16)  # [h', t*W + j]
        for t in range(2):
            for jh in range(2):
                eng = nc.scalar if t == 0 else nc.vector
                eng.tensor_copy(
                    kT_sb[:, t * W + jh * HW2 : t * W + jh * HW2 + HW2],
                    kT_ps[:, t * 1024 + jh * 512 : t * 1024 + jh * 512 + HW2],
                ) if t == 1 else nc.scalar.copy(
                    kT_sb[:, t * W + jh * HW2 : t * W + jh * HW2 + HW2],
                    kT_ps[:, t * 1024 + jh * 512 : t * 1024 + jh * 512 + HW2],
                )

        qT_ps = ps_sm.tile([128, 2 * 128], F32, tag="qt")
        for t in range(2):
            for dt in range(2):
                nc.tensor.matmul(
                    qT_ps[:, ts(t, 128)],
                    Wq_sb[:, dt * H + t * 128 : dt * H + t * 128 + 128],
                    qsT_sb[:, ts(dt, 128)],
                    start=(dt == 0),
                    stop=(dt == 1),
                )
        qT_sb = proj.tile([128, 2 * 128], F32)  # [h', t*128 + u*64 + l]
        nc.vector.tensor_copy(qT_sb[:], qT_ps[:])

        # ---- scores accumulator; per-row masked init (identity matmul) ----
        sc_ps = ps_sc.tile([128, W], F32)
        for jh in range((W + 511) // 512):
            hi = min(W, jh * 512 + 512)
            nc.tensor.matmul(
                sc_ps[:, jh * 512 : hi], identb_sb[:], mask_sb[:, jh * 512 : hi],
                start=True, stop=False, skip_group_check=True,
            )

        # ---- main loop ----
        # per unit u: preact tiles cover nl rows x (unit slot width), row-
        # interleaved so each DVE tensor_scalar_add spans the unit's full
        # slot range (FD = width) and each ACT tanh call is one long N
        def emit_unit(u, t, l0, nl, split):
            base = 0 if u == 0 else WA
            wcols = WA if u == 0 else WB
            pre = prep.tile([128, nl * wcols], BF16, tag="pre")
            for l in range(l0, l0 + nl):
                nc.vector.tensor_scalar_add(
                    pre[:, (l - l0) * wcols : (l - l0 + 1) * wcols],
                    kT_sb[:, t * W + base : t * W + base + wcols],
                    qT_sb[:, t * 128 + u * 64 + l : t * 128 + u * 64 + l + 1],
                )
            feat = featp.tile([128, nl * wcols], BF16, tag="feat")
            if split:
                nq = nl * wcols // 4
                for q4 in range(4):
                    nc.scalar.activation(
                        feat[:, ts(q4, nq)], pre[:, ts(q4, nq)], AF.Tanh
                    )
            else:
                nc.scalar.activation(feat[:], pre[:], AF.Tanh)
            for s in range(wcols // JB):
                for l in range(l0, l0 + nl):
                    g = l % 4
                    nc.tensor.matmul(
                        sc_ps[32 * g : 32 * g + 32, base + s * JB : base + (s + 1) * JB],
                        w32_sb[:, (u * 2 + t) * 2048 + l * 32 : (u * 2 + t) * 2048 + l * 32 + 32],
                        feat[:, (l - l0) * wcols + s * JB : (l - l0) * wcols + s * JB + JB],
                        start=False,
                        stop=False,
                        tile_position=(0, 32 * g),
                        skip_group_check=True,
                    )

        for t in range(2):
            for lh in range(2):
                emit_unit(0, t, 32 * lh, 32, split=(t == 0 and lh == 0))

        # A-unit score columns [0:WA) are final once the A accumulates are
        # done; run their softmax-exp + attn@values during the B groups.
        p_sb = sm.tile([128, W], BF16)
        se = sm.tile([128, 2], F32)
        nc.scalar.activation(
            p_sb[:, 0:WA], sc_ps[:, 0:WA], AF.Exp, accum_out=se[:, 0:1]
        )
        pT_ps = ps_big.tile([128, NJ6 * 128], BF16, tag="big")
        pT_sb = sm.tile([128, NJ6 * 128], BF16)  # [j', jb*128 + i]
        out_ps = ps_sm.tile([128, H], F32, tag="qt")
        NJA = WA // 128
        for jb in range(NJA):
            nc.tensor.transpose(
                pT_ps[:, ts(jb, 128)], p_sb[:, ts(jb, 128)], identb_sb[:]
            )
        nc.vector.tensor_copy(pT_sb[:, 0 : NJA * 128], pT_ps[:, 0 : NJA * 128])
        for jb in range(NJA):
            nc.tensor.matmul(
                out_ps[:],
                pT_sb[:, ts(jb, 128)],
                values_r[:, ts(jb, H)],
                start=(jb == 0),
                stop=False,
            )

        if SB == 1:
            emit_unit(1, 0, 0, 64, split=False)
            emit_unit(1, 1, 0, 64, split=True)
        else:
            emit_unit(1, 0, 0, 32, split=False)
            emit_unit(1, 0, 32, 32, split=False)
            emit_unit(1, 1, 0, 32, split=False)
            emit_unit(1, 1, 32, 32, split=True)

        # B-unit tail: exp + transposes + final accumulating matmuls
        nc.scalar.activation(
            p_sb[:, WA:W], sc_ps[:, WA:W], AF.Exp, accum_out=se[:, 1:2]
        )
        for jb in range(NJA, NJ6):
            nc.tensor.transpose(
                pT_ps[:, ts(jb, 128)], p_sb[:, ts(jb, 128)], identb_sb[:]
            )
        nc.vector.tensor_copy(
            pT_sb[:, NJA * 128 : NJ6 * 128], pT_ps[:, NJA * 128 : NJ6 * 128]
        )
        sumexp = sm.tile([128, 1], F32)
        nc.vector.tensor_add(sumexp[:], se[:, 0:1], se[:, 1:2])
        rinv = sm.tile([128, 1], F32)
        nc.vector.reciprocal(rinv[:], sumexp[:])
        for jb in range(NJA, NJ6):
            nc.tensor.matmul(
                out_ps[:],
                pT_sb[:, ts(jb, 128)],
                values_r[:, ts(jb, H)],
                start=False,
                stop=(jb == NJ6 - 1),
            )

        # (softmax + attn@values emitted interleaved with the B groups above)
        out_sb = sm.tile([128, H], F32)
        nc.vector.tensor_scalar_mul(out_sb[:], out_ps[:], rinv[:])
        nc.sync.dma_start(out_d[:], out_sb[:])

    nc.compile()
    return nc


def _get_program(key):
    if key not in _CACHE:
        _CACHE[key] = build_program(*key)
    return _CACHE[key]


def make_schedule(valid_lens):
    """Pack 16 (batch, row-half) units, sizes ceil(vl/256), two per core
    (largest-with-smallest pairing). Returns (SA, SB, schedule) where
    schedule[core] = ((bA, halfA, jbA), (bB, halfB, jbB))."""
    vl = np.asarray(valid_lens).astype(np.int64).reshape(B)
    jb = [min(Lk // JB, max(1, int(-(-v // JB)))) for v in vl]
    units = [(b, h, jb[b]) for b in range(B) for h in range(2)]
    order = sorted(range(16), key=lambda idx: -units[idx][2])
    pairs = [(units[order[k]], units[order[15 - k]]) for k in range(8)]
    SA = max(p[0][2] for p in pairs)
    SB = max(p[1][2] for p in pairs)
    return SA, SB, pairs


def make_in_maps(queries, keys, values, valid_lens, Wq, Wk, wv):
    queries = np.ascontiguousarray(queries, dtype=np.float32)
    keys = np.ascontiguousarray(keys, dtype=np.float32)
    values = np.ascontiguousarray(values, dtype=np.float32)
    Wq = np.ascontiguousarray(Wq, dtype=np.float32)
    Wk = np.ascontiguousarray(Wk, dtype=np.float32)
    wv = np.ascontiguousarray(wv, dtype=np.float32).reshape(H)
    vl = np.asarray(valid_lens).astype(np.int64).reshape(B)
    SA, SB, schedule = make_schedule(vl)
    S = SA + SB
    W = S * JB
    bf = ml_dtypes.bfloat16
    ident = np.eye(128, dtype=np.float32)
    identb = np.eye(128, dtype=bf)
    wvb = wv.astype(bf)
    # shared one-hot wv stationaries: block (u, t)
    wv32 = np.zeros((2, 2, 128, 64, 32), dtype=bf)
    ll = np.arange(64)
    for u in range(2):
        for t in range(2):
            wv32[u, t, :, ll, 16 * u + ll // 4] = wvb[t * 128 : (t + 1) * 128]
    wv32_pm = np.ascontiguousarray(
        wv32.reshape(4, 128, 64 * 32).transpose(1, 0, 2).reshape(128, -1)
    )
    Wq_pm = np.ascontiguousarray(
        Wq.reshape(2, 128, H).transpose(1, 0, 2).reshape(128, 2 * H)
    ).astype(bf)
    Wk_pm = np.ascontiguousarray(
        Wk.reshape(2, 128, H).transpose(1, 0, 2).reshape(128, 2 * H)
    ).astype(bf)
    jj = np.arange(JB)
    in_maps = []
    for core in range(NCORES):
        uA, uB = schedule[core]
        keysT_c = np.zeros((D, W), dtype=np.float32)
        values_c = np.zeros((W, H), dtype=np.float32)
        mask_c = np.full((128, W), -1e6, dtype=np.float32)
        qstack = np.zeros((128, D), dtype=np.float32)
        for u, (b, half, jbu), s0, su in ((0, uA, 0, SA), (1, uB, SA, SB)):
            qstack[u * 64 : u * 64 + 64, :] = queries[b, half * 64 : half * 64 + 64, :]
            rows = np.array([i_phys(u, l) for l in range(64)])
            for k in range(min(jbu, su)):
                s = s0 + k
                keysT_c[:, s * JB : (s + 1) * JB] = keys[b, k * JB : (k + 1) * JB, :].T
                values_c[s * JB : (s + 1) * JB, :] = values[b, k * JB : (k + 1) * JB, :]
                valid = np.minimum(np.maximum(vl[b] - k * JB, 0), JB)
                mask_c[rows[:, None], s * JB + jj[None, :]] = np.where(
                    (jj < valid)[None, :], 0.0, -1e6
                )
        qsT_pm = np.ascontiguousarray(
            qstack.T.reshape(2, 128, 128).transpose(1, 0, 2).reshape(128, D)
        )
        keysT_pm = np.ascontiguousarray(
            keysT_c.reshape(2, 128, W).transpose(1, 0, 2).reshape(128, 2 * W)
        )
        values_pm = np.ascontiguousarray(
            values_c.reshape(W // 128, 128, H).transpose(1, 0, 2).reshape(128, -1)
        )
        in_maps.append(
            {
                "qsT": qsT_pm.astype(bf),
                "keysT": keysT_pm.astype(bf),
                "values": values_pm.astype(bf),
                "Wq": Wq_pm,
                "Wk": Wk_pm,
                "mask": mask_c.astype(bf),
                "identb": identb,
                "ident": ident,
                "wv32": wv32_pm,
            }
        )
    return (SA, SB), schedule, in_maps


def assemble(schedule, core_outs):
    out = np.zeros((B, Lq, H), dtype=np.float32)
    for core in range(NCORES):
        uA, uB = schedule[core]
        oc = core_outs[core]
        for u, (b, half, _) in ((0, uA), (1, uB)):
            for l in range(64):
                out[b, half * 64 + l, :] = oc[i_phys(u, l), :]
    return out


def kernel(**inputs):
    key, schedule, in_maps = make_in_maps(
        inputs["queries"],
        inputs["keys"],
        inputs["values"],
        inputs["valid_lens"],
        inputs["Wq"],
        inputs["Wk"],
        inputs["wv"],
    )
    nc = _get_program(key)
    res = run_bass_kernel_spmd(nc, in_maps, core_ids=list(range(NCORES)))
    return assemble(schedule, [res.results[c]["out"] for c in range(NCORES)])


# revision 32
# speedup vs baseline: 1.0368x; 1.0368x over previous
"""AdditiveAttention on Trainium2 (Bass/Tile), 8 cores, valid_lens-aware resharding.

Reference per batch b:
  q = queries @ Wq; k = keys @ Wk
  scores[i,j] = wv . tanh(q[i] + k[j]); masked softmax over j; out = attn @ values

Masked columns (j >= valid_len) contribute exactly zero, so only
ceil(valid_len/256) 256-wide j-slots per batch need computing. Work units of
(batch, 64 query rows) are packed two per core (unit A -> u=0, unit B -> u=1)
with S_A slots for A and S_B for B (largest-with-smallest pairing; short
units get zero-key, -1e6-masked pad slots). One SPMD program: all per-core
variation lives in input DATA (slot-gathered keys/values/mask, stacked
queries); instructions are identical on every core.

Row permutation i_phys(u, l) = 32*(l%4) + 16*u + l//4 puts consecutive l on
disjoint PE column groups (4-way tile_position concurrency); the one-hot
column of the wv stationary (shared constant data) routes each matmul's
result to that row.

Engine split: PE projects kT/qT; DVE builds preact via tensor_scalar_add
(per-partition scalar q column, FD = S_A*256 per call via slot-interleaved
layout); ACT does tanh in 2*(2+S_B... ) long N=16K calls (the bottleneck)
plus the final exp (no max-subtraction: |scores| <= sum|wv| ~ 13); PE
accumulates scores via one-hot wv matmuls and computes attn @ values.
"""

import numpy as np
import ml_dtypes
from contextlib import ExitStack

from concourse import bacc, tile
import concourse.bass as bass
import concourse.mybir as mybir
from concourse.bass_utils import run_bass_kernel_spmd

F32 = mybir.dt.float32
F32R = mybir.dt.float32r
BF16 = mybir.dt.bfloat16
AF = mybir.ActivationFunctionType
ts = bass.ts

B, Lq, Lk, D, H = 8, 128, 512, 256, 256
NCORES = 8
JB = 256           # j-slot width

_CACHE = {}


def i_phys(u, l):
    return 32 * (l % 4) + 16 * u + l // 4


def build_program(SA, SB):
    nc = bacc.Bacc(
        "TRN2", target_bir_lowering=False, debug=False, enable_asserts=False
    )

    S = SA + SB
    W = S * JB          # gathered scores width
    WA, WB = SA * JB, SB * JB
    qsT_d = nc.dram_tensor("qsT", [128, D], BF16, kind="ExternalInput")
    keysT_d = nc.dram_tensor("keysT", [128, 2 * W], BF16, kind="ExternalInput")
    Wq_d = nc.dram_tensor("Wq", [128, 2 * H], BF16, kind="ExternalInput")
    Wk_d = nc.dram_tensor("Wk", [128, 2 * H], BF16, kind="ExternalInput")
    mask_d = nc.dram_tensor("mask", [128, W], BF16, kind="ExternalInput")
    identb_d = nc.dram_tensor("identb", [128, 128], BF16, kind="ExternalInput")
    ident_d = nc.dram_tensor("ident", [128, 128], F32, kind="ExternalInput")
    # wv32[(u*2+t)*128 + k, l*32 + r] = wv[t*128+k] iff r == 16*u + l//4
    wv32_d = nc.dram_tensor("wv32", [128, 4 * 64 * 32], BF16, kind="ExternalInput")
    out_d = nc.dram_tensor("out", [Lq, H], F32, kind="ExternalOutput")

    NJ6 = W // 128      # 128-row j-blocks of the gathered axis (for attn@values)
    values_d = nc.dram_tensor("values", [128, NJ6 * H], BF16, kind="ExternalInput")

    with tile.TileContext(nc) as tc, ExitStack() as ctx:
        const = ctx.enter_context(tc.tile_pool(name="const", bufs=1))
        inp = ctx.enter_context(tc.tile_pool(name="inp", bufs=1))
        proj = ctx.enter_context(tc.tile_pool(name="proj", bufs=1))
        prep = ctx.enter_context(tc.tile_pool(name="prep", bufs=2))
        featp = ctx.enter_context(tc.tile_pool(name="featp", bufs=2))
        sm = ctx.enter_context(tc.tile_pool(name="sm", bufs=1))
        ps_big = ctx.enter_context(tc.tile_pool(name="ps_big", bufs=1, space="PSUM"))
        ps_sc = ctx.enter_context(tc.tile_pool(name="ps_sc", bufs=1, space="PSUM"))
        ps_sm = ctx.enter_context(tc.tile_pool(name="ps_sm", bufs=1, space="PSUM"))

        # ---- ACT spline table warmup (tanh/exp share a set); no DMA dep ----
        warm_in = sm.tile([1, 2], F32)
        nc.vector.memset(warm_in[:], 0.0)
        warm_sb = sm.tile([1, 2], F32)
        nc.scalar.activation(warm_sb[0:1, 0:1], warm_in[0:1, 0:1], AF.Tanh)
        nc.scalar.activation(warm_sb[0:1, 1:2], warm_in[0:1, 0:1], AF.Exp)

        # ---- input loads; k-projection path first (it gates the pipeline) ----
        HW2 = W // 2
        qsT_sb = inp.tile([128, D], BF16)  # [d', dt*128 + (u*64+l)]
        nc.scalar.dma_start(qsT_sb[:], qsT_d[:])
        Wq_sb = inp.tile([128, 2 * H], BF16)
        nc.scalar.dma_start(Wq_sb[:], Wq_d[:])
        keysT_sb = inp.tile([128, 2 * W], BF16)  # [d', dt*W + j]
        nc.sync.dma_start(keysT_sb[:], keysT_d[:])
        Wk_sb = inp.tile([128, 2 * H], BF16)  # [d', dt*256 + h]
        nc.gpsimd.dma_start(Wk_sb[:], Wk_d[:])
        mask_sb = const.tile([128, W], BF16)
        nc.gpsimd.dma_start(mask_sb[:], mask_d[:])
        identb_sb = const.tile([128, 128], BF16)
        nc.sync.dma_start(identb_sb[:], identb_d[:])
        ident_sb = const.tile([128, 128], F32)
        nc.sync.dma_start(ident_sb[:], ident_d[:])
        w32_sb = const.tile([128, 4 * 64 * 32], BF16)  # [(u*2+t) blocks]
        nc.gpsimd.dma_start(w32_sb[:], wv32_d[:])
        values_r = inp.tile([128, NJ6 * H], BF16)  # [j', jb*256 + v]
        nc.gpsimd.dma_start(values_r[:], values_d[:])

        # ---- projections (q first: its inputs land earlier) ----
        qT_ps = ps_sm.tile([128, 2 * 128], F32, tag="qt")
        for t in range(2):
            for dt in range(2):
                nc.tensor.matmul(
                    qT_ps[:, ts(t, 128)],
                    Wq_sb[:, dt * H + t * 128 : dt * H + t * 128 + 128],
                    qsT_sb[:, ts(dt, 128)],
                    start=(dt == 0),
                    stop=(dt == 1),
                )
        qT_sb = proj.tile([128, 2 * 128], F32)  # [h', t*128 + u*64 + l]
        nc.vector.tensor_copy(qT_sb[:], qT_ps[:])

        # kT_ps laid out [t*1024 + jh*512 .. +W/2] so no matmul output
        # crosses a PSUM bank boundary
        kT_ps = ps_big.tile([128, 2048], F32, tag="big")
        for t in range(2):
            for jh in range(2):
                for dt in range(2):
                    nc.tensor.matmul(
                        kT_ps[:, t * 1024 + jh * 512 : t * 1024 + jh * 512 + HW2],
                        Wk_sb[:, dt * H + t * 128 : dt * H + t * 128 + 128],
                        keysT_sb[:, dt * W + jh * HW2 : dt * W + jh * HW2 + HW2],
                        start=(dt == 0),
                        stop=(dt == 1),
                    )
        kT_sb = proj.tile([128, 2 * W], BF16)  # [h', t*W + j]
        for t in range(2):
            for jh in range(2):
                eng = nc.scalar if t == 0 else nc.vector
                eng.tensor_copy(
                    kT_sb[:, t * W + jh * HW2 : t * W + jh * HW2 + HW2],
                    kT_ps[:, t * 1024 + jh * 512 : t * 1024 + jh * 512 + HW2],
                ) if t == 1 else nc.scalar.copy(
                    kT_sb[:, t * W + jh * HW2 : t * W + jh * HW2 + HW2],
                    kT_ps[:, t * 1024 + jh * 512 : t * 1024 + jh * 512 + HW2],
                )

        # ---- scores accumulator; per-row masked init (identity matmul) ----
        sc_ps = ps_sc.tile([128, W], F32)
        for jh in range((W + 511) // 512):
            hi = min(W, jh * 512 + 512)
            nc.tensor.matmul(
                sc_ps[:, jh * 512 : hi], identb_sb[:], mask_sb[:, jh * 512 : hi],
                start=True, stop=False, skip_group_check=True,
            )

        # ---- main loop ----
        # per unit u: preact tiles cover nl rows x (unit slot width), row-
        # interleaved so each DVE tensor_scalar_add spans the unit's full
        # slot range (FD = width) and each ACT tanh call is one long N
        def emit_unit(u, t, l0, nl, split):
            base = 0 if u == 0 else WA
            wcols = WA if u == 0 else WB
            pre = prep.tile([128, nl * wcols], BF16, tag="pre")
            for l in range(l0, l0 + nl):
                nc.vector.tensor_scalar_add(
                    pre[:, (l - l0) * wcols : (l - l0 + 1) * wcols],
                    kT_sb[:, t * W + base : t * W + base + wcols],
                    qT_sb[:, t * 128 + u * 64 + l : t * 128 + u * 64 + l + 1],
                )
            feat = featp.tile([128, nl * wcols], BF16, tag="feat")
            if split:
                nq = nl * wcols // 4
                for q4 in range(4):
                    nc.scalar.activation(
                        feat[:, ts(q4, nq)], pre[:, ts(q4, nq)], AF.Tanh
                    )
            else:
                nc.scalar.activation(feat[:], pre[:], AF.Tanh)
            for s in range(wcols // JB):
                for l in range(l0, l0 + nl):
                    g = l % 4
                    nc.tensor.matmul(
                        sc_ps[32 * g : 32 * g + 32, base + s * JB : base + (s + 1) * JB],
                        w32_sb[:, (u * 2 + t) * 2048 + l * 32 : (u * 2 + t) * 2048 + l * 32 + 32],
                        feat[:, (l - l0) * wcols + s * JB : (l - l0) * wcols + s * JB + JB],
                        start=False,
                        stop=False,
                        tile_position=(0, 32 * g),
                        skip_group_check=True,
                    )

        for t in range(2):
            for lh in range(2):
                emit_unit(0, t, 32 * lh, 32, split=(t == 0 and lh == 0))

        # A-unit score columns [0:WA) are final once the A accumulates are
        # done; run their softmax-exp + attn@values during the B groups.
        p_sb = sm.tile([128, W], BF16)
        se = sm.tile([128, 2], F32)
        nc.scalar.activation(
            p_sb[:, 0:WA], sc_ps[:, 0:WA], AF.Exp, accum_out=se[:, 0:1]
        )
        pT_ps = ps_big.tile([128, NJ6 * 128], BF16, tag="big")
        pT_sb = sm.tile([128, NJ6 * 128], BF16)  # [j', jb*128 + i]
        out_ps = ps_sm.tile([128, H], F32, tag="qt")
        NJA = WA // 128
        for jb in range(NJA):
            nc.tensor.transpose(
                pT_ps[:, ts(jb, 128)], p_sb[:, ts(jb, 128)], identb_sb[:]
            )
        nc.vector.tensor_copy(pT_sb[:, 0 : NJA * 128], pT_ps[:, 0 : NJA * 128])
        for jb in range(NJA):
            nc.tensor.matmul(
                out_ps[:],
                pT_sb[:, ts(jb, 128)],
                values_r[:, ts(jb, H)],
                start=(jb == 0),
                stop=False,
            )

        if SB == 1:
            emit_unit(1, 0, 0, 64, split=False)
            emit_unit(1, 1, 0, 64, split=True)
        else:
            emit_unit(1, 0, 0, 32, split=False)
            emit_unit(1, 0, 32, 32, split=False)
            emit_unit(1, 1, 0, 32, split=False)
            emit_unit(1, 1, 32, 32, split=True)

        # B-unit tail: exp + transposes + final accumulating matmuls
        nc.scalar.activation(
            p_sb[:, WA:W], sc_ps[:, WA:W], AF.Exp, accum_out=se[:, 1:2]
        )
        for jb in range(NJA, NJ6):
            nc.tensor.transpose(
                pT_ps[:, ts(jb, 128)], p_sb[:, ts(jb, 128)], identb_sb[:]
            )
        nc.vector.tensor_copy(
            pT_sb[:, NJA * 128 : NJ6 * 128], pT_ps[:, NJA * 128 : NJ6 * 128]
        )
        sumexp = sm.tile([128, 1], F32)
        nc.vector.tensor_add(sumexp[:], se[:, 0:1], se[:, 1:2])
        rinv = sm.tile([128, 1], F32)
        nc.vector.reciprocal(rinv[:], sumexp[:])
        for jb in range(NJA, NJ6):
            nc.tensor.matmul(
                out_ps[:],
                pT_sb[:, ts(jb, 128)],
                values_r[:, ts(jb, H)],
                start=False,
                stop=(jb == NJ6 - 1),
            )

        # (softmax + attn@values emitted interleaved with the B groups above)
        out_sb = sm.tile([128, H], F32)
        nc.vector.tensor_scalar_mul(out_sb[:], out_ps[:], rinv[:])
        nc.sync.dma_start(out_d[:], out_sb[:])

    nc.compile()
    return nc


def _get_program(key):
    if key not in _CACHE:
        _CACHE[key] = build_program(*key)
    return _CACHE[key]


def make_schedule(valid_lens):
    """Pack 16 (batch, row-half) units, sizes ceil(vl/256), two per core
    (largest-with-smallest pairing). Returns (SA, SB, schedule) where
    schedule[core] = ((bA, halfA, jbA), (bB, halfB, jbB))."""
    vl = np.asarray(valid_lens).astype(np.int64).reshape(B)
    jb = [min(Lk // JB, max(1, int(-(-v // JB)))) for v in vl]
    units = [(b, h, jb[b]) for b in range(B) for h in range(2)]
    order = sorted(range(16), key=lambda idx: -units[idx][2])
    pairs = [(units[order[k]], units[order[15 - k]]) for k in range(8)]
    SA = max(p[0][2] for p in pairs)
    SB = max(p[1][2] for p in pairs)
    return SA, SB, pairs


def make_in_maps(queries, keys, values, valid_lens, Wq, Wk, wv):
    queries = np.ascontiguousarray(queries, dtype=np.float32)
    keys = np.ascontiguousarray(keys, dtype=np.float32)
    values = np.ascontiguousarray(values, dtype=np.float32)
    Wq = np.ascontiguousarray(Wq, dtype=np.float32)
    Wk = np.ascontiguousarray(Wk, dtype=np.float32)
    wv = np.ascontiguousarray(wv, dtype=np.float32).reshape(H)
    vl = np.asarray(valid_lens).astype(np.int64).reshape(B)
    SA, SB, schedule = make_schedule(vl)
    S = SA + SB
    W = S * JB
    bf = ml_dtypes.bfloat16
    ident = np.eye(128, dtype=np.float32)
    identb = np.eye(128, dtype=bf)
    wvb = wv.astype(bf)
    # shared one-hot wv stationaries: block (u, t)
    wv32 = np.zeros((2, 2, 128, 64, 32), dtype=bf)
    ll = np.arange(64)
    for u in range(2):
        for t in range(2):
            wv32[u, t, :, ll, 16 * u + ll // 4] = wvb[t * 128 : (t + 1) * 128]
    wv32_pm = np.ascontiguousarray(
        wv32.reshape(4, 128, 64 * 32).transpose(1, 0, 2).reshape(128, -1)
    )
    Wq_pm = np.ascontiguousarray(
        Wq.reshape(2, 128, H).transpose(1, 0, 2).reshape(128, 2 * H)
    ).astype(bf)
    Wk_pm = np.ascontiguousarray(
        Wk.reshape(2, 128, H).transpose(1, 0, 2).reshape(128, 2 * H)
    ).astype(bf)
    jj = np.arange(JB)
    in_maps = []
    for core in range(NCORES):
        uA, uB = schedule[core]
        keysT_c = np.zeros((D, W), dtype=np.float32)
        values_c = np.zeros((W, H), dtype=np.float32)
        mask_c = np.full((128, W), -1e6, dtype=np.float32)
        qstack = np.zeros((128, D), dtype=np.float32)
        for u, (b, half, jbu), s0, su in ((0, uA, 0, SA), (1, uB, SA, SB)):
            qstack[u * 64 : u * 64 + 64, :] = queries[b, half * 64 : half * 64 + 64, :]
            rows = np.array([i_phys(u, l) for l in range(64)])
            for k in range(min(jbu, su)):
                s = s0 + k
                keysT_c[:, s * JB : (s + 1) * JB] = keys[b, k * JB : (k + 1) * JB, :].T
                values_c[s * JB : (s + 1) * JB, :] = values[b, k * JB : (k + 1) * JB, :]
                valid = np.minimum(np.maximum(vl[b] - k * JB, 0), JB)
                mask_c[rows[:, None], s * JB + jj[None, :]] = np.where(
                    (jj < valid)[None, :], 0.0, -1e6
                )
        qsT_pm = np.ascontiguousarray(
            qstack.T.reshape(2, 128, 128).transpose(1, 0, 2).reshape(128, D)
        )
        keysT_pm = np.ascontiguousarray(
            keysT_c.reshape(2, 128, W).transpose(1, 0, 2).reshape(128, 2 * W)
        )
        values_pm = np.ascontiguousarray(
            values_c.reshape(W // 128, 128, H).transpose(1, 0, 2).reshape(128, -1)
        )
        in_maps.append(
            {
                "qsT": qsT_pm.astype(bf),
                "keysT": keysT_pm.astype(bf),
                "values": values_pm.astype(bf),
                "Wq": Wq_pm,
                "Wk": Wk_pm,
                "mask": mask_c.astype(bf),
                "identb": identb,
                "ident": ident,
                "wv32": wv32_pm,
            }
        )
    return (SA, SB), schedule, in_maps


def assemble(schedule, core_outs):
    out = np.zeros((B, Lq, H), dtype=np.float32)
    for core in range(NCORES):
        uA, uB = schedule[core]
        oc = core_outs[core]
        for u, (b, half, _) in ((0, uA), (1, uB)):
            for l in range(64):
                out[b, half * 64 + l, :] = oc[i_phys(u, l), :]
    return out


def kernel(**inputs):
    key, schedule, in_maps = make_in_maps(
        inputs["queries"],
        inputs["keys"],
        inputs["values"],
        inputs["valid_lens"],
        inputs["Wq"],
        inputs["Wk"],
        inputs["wv"],
    )
    nc = _get_program(key)
    res = run_bass_kernel_spmd(nc, in_maps, core_ids=list(range(NCORES)))
    return assemble(schedule, [res.results[c]["out"] for c in range(NCORES)])


# revision 33
# speedup vs baseline: 1.0490x; 1.0117x over previous
"""AdditiveAttention on Trainium2 (Bass/Tile), 8 cores, valid_lens-aware resharding.

Reference per batch b:
  q = queries @ Wq; k = keys @ Wk
  scores[i,j] = wv . tanh(q[i] + k[j]); masked softmax over j; out = attn @ values

Masked columns (j >= valid_len) contribute exactly zero, so only
ceil(valid_len/256) 256-wide j-slots per batch need computing. Work units of
(batch, 64 query rows) are packed two per core (unit A -> u=0, unit B -> u=1)
with S_A slots for A and S_B for B (largest-with-smallest pairing; short
units get zero-key, -1e6-masked pad slots). One SPMD program: all per-core
variation lives in input DATA (slot-gathered keys/values/mask, stacked
queries); instructions are identical on every core.

Row permutation i_phys(u, l) = 32*(l%4) + 16*u + l//4 puts consecutive l on
disjoint PE column groups (4-way tile_position concurrency); the one-hot
column of the wv stationary (shared constant data) routes each matmul's
result to that row.

Engine split: PE projects kT/qT; DVE builds preact via tensor_scalar_add
(per-partition scalar q column, FD = S_A*256 per call via slot-interleaved
layout); ACT does tanh in 2*(2+S_B... ) long N=16K calls (the bottleneck)
plus the final exp (no max-subtraction: |scores| <= sum|wv| ~ 13); PE
accumulates scores via one-hot wv matmuls and computes attn @ values.
"""

import numpy as np
import ml_dtypes
from contextlib import ExitStack

from concourse import bacc, tile
import concourse.bass as bass
import concourse.mybir as mybir
from concourse.bass_utils import run_bass_kernel_spmd

F32 = mybir.dt.float32
F32R = mybir.dt.float32r
BF16 = mybir.dt.bfloat16
AF = mybir.ActivationFunctionType
ts = bass.ts

B, Lq, Lk, D, H = 8, 128, 512, 256, 256
NCORES = 8
JB = 256           # j-slot width

_CACHE = {}


def i_phys(u, l):
    return 32 * (l % 4) + 16 * u + l // 4


def build_program(SA, SB):
    nc = bacc.Bacc(
        "TRN2", target_bir_lowering=False, debug=False, enable_asserts=False
    )

    S = SA + SB
    W = S * JB          # gathered scores width
    WA, WB = SA * JB, SB * JB
    qsT_d = nc.dram_tensor("qsT", [128, D], BF16, kind="ExternalInput")
    keysT_d = nc.dram_tensor("keysT", [128, 2 * W], BF16, kind="ExternalInput")
    Wq_d = nc.dram_tensor("Wq", [128, 2 * H], BF16, kind="ExternalInput")
    Wk_d = nc.dram_tensor("Wk", [128, 2 * H], BF16, kind="ExternalInput")
    mask_d = nc.dram_tensor("mask", [128, W], BF16, kind="ExternalInput")
    identb_d = nc.dram_tensor("identb", [128, 128], BF16, kind="ExternalInput")
    ident_d = nc.dram_tensor("ident", [128, 128], F32, kind="ExternalInput")
    # wv32[(u*2+t)*128 + k, l*32 + r] = wv[t*128+k] iff r == 16*u + l//4
    wv32_d = nc.dram_tensor("wv32", [128, 4 * 64 * 32], BF16, kind="ExternalInput")
    out_d = nc.dram_tensor("out", [Lq, H], F32, kind="ExternalOutput")

    NJ6 = W // 128      # 128-row j-blocks of the gathered axis (for attn@values)
    values_d = nc.dram_tensor("values", [128, NJ6 * H], BF16, kind="ExternalInput")

    with tile.TileContext(nc) as tc, ExitStack() as ctx:
        const = ctx.enter_context(tc.tile_pool(name="const", bufs=1))
        inp = ctx.enter_context(tc.tile_pool(name="inp", bufs=1))
        proj = ctx.enter_context(tc.tile_pool(name="proj", bufs=1))
        prep = ctx.enter_context(tc.tile_pool(name="prep", bufs=2))
        featp = ctx.enter_context(tc.tile_pool(name="featp", bufs=2))
        sm = ctx.enter_context(tc.tile_pool(name="sm", bufs=1))
        ps_big = ctx.enter_context(tc.tile_pool(name="ps_big", bufs=1, space="PSUM"))
        ps_sc = ctx.enter_context(tc.tile_pool(name="ps_sc", bufs=1, space="PSUM"))
        ps_sm = ctx.enter_context(tc.tile_pool(name="ps_sm", bufs=1, space="PSUM"))

        # ---- ACT spline table warmup (tanh/exp share a set); no DMA dep ----
        warm_in = sm.tile([1, 2], F32)
        nc.vector.memset(warm_in[:], 0.0)
        warm_sb = sm.tile([1, 2], F32)
        nc.scalar.activation(warm_sb[0:1, 0:1], warm_in[0:1, 0:1], AF.Tanh)
        nc.scalar.activation(warm_sb[0:1, 1:2], warm_in[0:1, 0:1], AF.Exp)

        # ---- input loads; k-projection path first (it gates the pipeline) ----
        HW2 = W // 2
        qsT_sb = inp.tile([128, D], BF16)  # [d', dt*128 + (u*64+l)]
        nc.scalar.dma_start(qsT_sb[:], qsT_d[:])
        Wq_sb = inp.tile([128, 2 * H], BF16)
        nc.scalar.dma_start(Wq_sb[:], Wq_d[:])
        keysT_sb = inp.tile([128, 2 * W], BF16)  # [d', dt*W + j]
        nc.sync.dma_start(keysT_sb[:], keysT_d[:])
        Wk_sb = inp.tile([128, 2 * H], BF16)  # [d', dt*256 + h]
        nc.gpsimd.dma_start(Wk_sb[:], Wk_d[:])
        mask_sb = const.tile([128, W], BF16)
        nc.gpsimd.dma_start(mask_sb[:], mask_d[:])
        identb_sb = const.tile([128, 128], BF16)
        nc.sync.dma_start(identb_sb[:], identb_d[:])
        ident_sb = const.tile([128, 128], F32)
        nc.sync.dma_start(ident_sb[:], ident_d[:])
        w32_sb = const.tile([128, 4 * 64 * 32], BF16)  # [(u*2+t) blocks]
        nc.gpsimd.dma_start(w32_sb[:], wv32_d[:])
        values_r = inp.tile([128, NJ6 * H], BF16)  # [j', jb*256 + v]
        nc.gpsimd.dma_start(values_r[:], values_d[:])

        # ---- projections (q first: its inputs land earlier) ----
        qT_ps = ps_sm.tile([128, 2 * 128], F32, tag="qt")
        for t in range(2):
            for dt in range(2):
                nc.tensor.matmul(
                    qT_ps[:, ts(t, 128)],
                    Wq_sb[:, dt * H + t * 128 : dt * H + t * 128 + 128],
                    qsT_sb[:, ts(dt, 128)],
                    start=(dt == 0),
                    stop=(dt == 1),
                )
        qT_sb = proj.tile([128, 2 * 128], F32)  # [h', t*128 + u*64 + l]
        nc.vector.tensor_copy(qT_sb[:], qT_ps[:])

        # kT_ps laid out [t*1024 + jh*512 .. +W/2] so no matmul output
        # crosses a PSUM bank boundary
        kT_ps = ps_big.tile([128, 2048], F32, tag="big")
        for t in range(2):
            for jh in range(2):
                for dt in range(2):
                    nc.tensor.matmul(
                        kT_ps[:, t * 1024 + jh * 512 : t * 1024 + jh * 512 + HW2],
                        Wk_sb[:, dt * H + t * 128 : dt * H + t * 128 + 128],
                        keysT_sb[:, dt * W + jh * HW2 : dt * W + jh * HW2 + HW2],
                        start=(dt == 0),
                        stop=(dt == 1),
                    )
        kT_sb = proj.tile([128, 2 * W], BF16)  # [h', t*W + j]
        for t in range(2):
            for jh in range(2):
                eng = nc.scalar if t == 0 else nc.vector
                eng.tensor_copy(
                    kT_sb[:, t * W + jh * HW2 : t * W + jh * HW2 + HW2],
                    kT_ps[:, t * 1024 + jh * 512 : t * 1024 + jh * 512 + HW2],
                ) if t == 1 else nc.scalar.copy(
                    kT_sb[:, t * W + jh * HW2 : t * W + jh * HW2 + HW2],
                    kT_ps[:, t * 1024 + jh * 512 : t * 1024 + jh * 512 + HW2],
                )

        # ---- scores accumulator; per-row masked init (identity matmul) ----
        sc_ps = ps_sc.tile([128, W], F32)
        for jh in range((W + 511) // 512):
            hi = min(W, jh * 512 + 512)
            nc.tensor.matmul(
                sc_ps[:, jh * 512 : hi], identb_sb[:], mask_sb[:, jh * 512 : hi],
                start=True, stop=False, skip_group_check=True,
            )

        # ---- main loop ----
        # per unit u: preact tiles cover nl rows x (unit slot width), row-
        # interleaved so each DVE tensor_scalar_add spans the unit's full
        # slot range (FD = width) and each ACT tanh call is one long N
        def emit_unit(u, t, l0, nl, split):
            base = 0 if u == 0 else WA
            wcols = WA if u == 0 else WB
            pre = prep.tile([128, nl * wcols], BF16, tag="pre")
            for l in range(l0, l0 + nl):
                nc.vector.tensor_scalar_add(
                    pre[:, (l - l0) * wcols : (l - l0 + 1) * wcols],
                    kT_sb[:, t * W + base : t * W + base + wcols],
                    qT_sb[:, t * 128 + u * 64 + l : t * 128 + u * 64 + l + 1],
                )
            feat = featp.tile([128, nl * wcols], BF16, tag="feat")
            if split:
                nq = nl * wcols // 4
                for q4 in range(4):
                    nc.scalar.activation(
                        feat[:, ts(q4, nq)], pre[:, ts(q4, nq)], AF.Tanh
                    )
            else:
                nc.scalar.activation(feat[:], pre[:], AF.Tanh)
            for s in range(wcols // JB):
                for l in range(l0, l0 + nl):
                    g = l % 4
                    nc.tensor.matmul(
                        sc_ps[32 * g : 32 * g + 32, base + s * JB : base + (s + 1) * JB],
                        w32_sb[:, (u * 2 + t) * 2048 + l * 32 : (u * 2 + t) * 2048 + l * 32 + 32],
                        feat[:, (l - l0) * wcols + s * JB : (l - l0) * wcols + s * JB + JB],
                        start=False,
                        stop=False,
                        tile_position=(0, 32 * g),
                        skip_group_check=True,
                    )

        # Group A00: ACT is supply-bound at kernel start, so rows 0-3 are
        # computed by ACT alone (tanh with per-partition q bias fused, no DVE
        # preact dependency); the DVE concurrently builds rows 4-31, which
        # ACT consumes in three pieces.
        preA = prep.tile([128, 32 * WA], BF16, tag="pre")
        featA = featp.tile([128, 32 * WA], BF16, tag="feat")
        for l in range(4):
            nc.scalar.activation(
                featA[:, l * WA : (l + 1) * WA],
                kT_sb[:, 0:WA],
                AF.Tanh,
                bias=qT_sb[:, l : l + 1],
            )
        for l in range(4, 32):
            nc.vector.tensor_scalar_add(
                preA[:, l * WA : (l + 1) * WA],
                kT_sb[:, 0:WA],
                qT_sb[:, l : l + 1],
            )
        for lo, hi in ((4, 14), (14, 24), (24, 32)):
            nc.scalar.activation(
                featA[:, lo * WA : hi * WA], preA[:, lo * WA : hi * WA], AF.Tanh
            )
        for s in range(SA):
            for l in range(32):
                g = l % 4
                nc.tensor.matmul(
                    sc_ps[32 * g : 32 * g + 32, s * JB : (s + 1) * JB],
                    w32_sb[:, l * 32 : l * 32 + 32],
                    featA[:, l * WA + s * JB : l * WA + s * JB + JB],
                    start=False,
                    stop=False,
                    tile_position=(0, 32 * g),
                    skip_group_check=True,
                )
        emit_unit(0, 0, 32, 32, split=False)
        for lh in range(2):
            emit_unit(0, 1, 32 * lh, 32, split=False)

        # A-unit score columns [0:WA) are final once the A accumulates are
        # done; run their softmax-exp + attn@values during the B groups.
        p_sb = sm.tile([128, W], BF16)
        se = sm.tile([128, 2], F32)
        nc.scalar.activation(
            p_sb[:, 0:WA], sc_ps[:, 0:WA], AF.Exp, accum_out=se[:, 0:1]
        )
        pT_ps = ps_big.tile([128, NJ6 * 128], BF16, tag="big")
        pT_sb = sm.tile([128, NJ6 * 128], BF16)  # [j', jb*128 + i]
        out_ps = ps_sm.tile([128, H], F32, tag="qt")
        NJA = WA // 128
        for jb in range(NJA):
            nc.tensor.transpose(
                pT_ps[:, ts(jb, 128)], p_sb[:, ts(jb, 128)], identb_sb[:]
            )
        nc.vector.tensor_copy(pT_sb[:, 0 : NJA * 128], pT_ps[:, 0 : NJA * 128])
        for jb in range(NJA):
            nc.tensor.matmul(
                out_ps[:],
                pT_sb[:, ts(jb, 128)],
                values_r[:, ts(jb, H)],
                start=(jb == 0),
                stop=False,
            )

        if SB == 1:
            emit_unit(1, 0, 0, 64, split=False)
            emit_unit(1, 1, 0, 64, split=True)
        else:
            emit_unit(1, 0, 0, 32, split=False)
            emit_unit(1, 0, 32, 32, split=False)
            emit_unit(1, 1, 0, 32, split=False)
            emit_unit(1, 1, 32, 32, split=True)

        # B-unit tail: exp + transposes + final accumulating matmuls
        nc.scalar.activation(
            p_sb[:, WA:W], sc_ps[:, WA:W], AF.Exp, accum_out=se[:, 1:2]
        )
        for jb in range(NJA, NJ6):
            nc.tensor.transpose(
                pT_ps[:, ts(jb, 128)], p_sb[:, ts(jb, 128)], identb_sb[:]
            )
        nc.vector.tensor_copy(
            pT_sb[:, NJA * 128 : NJ6 * 128], pT_ps[:, NJA * 128 : NJ6 * 128]
        )
        sumexp = sm.tile([128, 1], F32)
        nc.vector.tensor_add(sumexp[:], se[:, 0:1], se[:, 1:2])
        rinv = sm.tile([128, 1], F32)
        nc.vector.reciprocal(rinv[:], sumexp[:])
        for jb in range(NJA, NJ6):
            nc.tensor.matmul(
                out_ps[:],
                pT_sb[:, ts(jb, 128)],
                values_r[:, ts(jb, H)],
                start=False,
                stop=(jb == NJ6 - 1),
            )

        # (softmax + attn@values emitted interleaved with the B groups above)
        out_sb = sm.tile([128, H], F32)
        nc.vector.tensor_scalar_mul(out_sb[:], out_ps[:], rinv[:])
        nc.sync.dma_start(out_d[:], out_sb[:])

    nc.compile()
    return nc


def _get_program(key):
    if key not in _CACHE:
        _CACHE[key] = build_program(*key)
    return _CACHE[key]


def make_schedule(valid_lens):
    """Pack 16 (batch, row-half) units, sizes ceil(vl/256), two per core
    (largest-with-smallest pairing). Returns (SA, SB, schedule) where
    schedule[core] = ((bA, halfA, jbA), (bB, halfB, jbB))."""
    vl = np.asarray(valid_lens).astype(np.int64).reshape(B)
    jb = [min(Lk // JB, max(1, int(-(-v // JB)))) for v in vl]
    units = [(b, h, jb[b]) for b in range(B) for h in range(2)]
    order = sorted(range(16), key=lambda idx: -units[idx][2])
    pairs = [(units[order[k]], units[order[15 - k]]) for k in range(8)]
    SA = max(p[0][2] for p in pairs)
    SB = max(p[1][2] for p in pairs)
    return SA, SB, pairs


def make_in_maps(queries, keys, values, valid_lens, Wq, Wk, wv):
    queries = np.ascontiguousarray(queries, dtype=np.float32)
    keys = np.ascontiguousarray(keys, dtype=np.float32)
    values = np.ascontiguousarray(values, dtype=np.float32)
    Wq = np.ascontiguousarray(Wq, dtype=np.float32)
    Wk = np.ascontiguousarray(Wk, dtype=np.float32)
    wv = np.ascontiguousarray(wv, dtype=np.float32).reshape(H)
    vl = np.asarray(valid_lens).astype(np.int64).reshape(B)
    SA, SB, schedule = make_schedule(vl)
    S = SA + SB
    W = S * JB
    bf = ml_dtypes.bfloat16
    ident = np.eye(128, dtype=np.float32)
    identb = np.eye(128, dtype=bf)
    wvb = wv.astype(bf)
    # shared one-hot wv stationaries: block (u, t)
    wv32 = np.zeros((2, 2, 128, 64, 32), dtype=bf)
    ll = np.arange(64)
    for u in range(2):
        for t in range(2):
            wv32[u, t, :, ll, 16 * u + ll // 4] = wvb[t * 128 : (t + 1) * 128]
    wv32_pm = np.ascontiguousarray(
        wv32.reshape(4, 128, 64 * 32).transpose(1, 0, 2).reshape(128, -1)
    )
    Wq_pm = np.ascontiguousarray(
        Wq.reshape(2, 128, H).transpose(1, 0, 2).reshape(128, 2 * H)
    ).astype(bf)
    Wk_pm = np.ascontiguousarray(
        Wk.reshape(2, 128, H).transpose(1, 0, 2).reshape(128, 2 * H)
    ).astype(bf)
    jj = np.arange(JB)
    in_maps = []
    for core in range(NCORES):
        uA, uB = schedule[core]
        keysT_c = np.zeros((D, W), dtype=np.float32)
        values_c = np.zeros((W, H), dtype=np.float32)
        mask_c = np.full((128, W), -1e6, dtype=np.float32)
        qstack = np.zeros((128, D), dtype=np.float32)
        for u, (b, half, jbu), s0, su in ((0, uA, 0, SA), (1, uB, SA, SB)):
            qstack[u * 64 : u * 64 + 64, :] = queries[b, half * 64 : half * 64 + 64, :]
            rows = np.array([i_phys(u, l) for l in range(64)])
            for k in range(min(jbu, su)):
                s = s0 + k
                keysT_c[:, s * JB : (s + 1) * JB] = keys[b, k * JB : (k + 1) * JB, :].T
                values_c[s * JB : (s + 1) * JB, :] = values[b, k * JB : (k + 1) * JB, :]
                valid = np.minimum(np.maximum(vl[b] - k * JB, 0), JB)
                mask_c[rows[:, None], s * JB + jj[None, :]] = np.where(
                    (jj < valid)[None, :], 0.0, -1e6
                )
        qsT_pm = np.ascontiguousarray(
            qstack.T.reshape(2, 128, 128).transpose(1, 0, 2).reshape(128, D)
        )
        keysT_pm = np.ascontiguousarray(
            keysT_c.reshape(2, 128, W).transpose(1, 0, 2).reshape(128, 2 * W)
        )
        values_pm = np.ascontiguousarray(
            values_c.reshape(W // 128, 128, H).transpose(1, 0, 2).reshape(128, -1)
        )
        in_maps.append(
            {
                "qsT": qsT_pm.astype(bf),
                "keysT": keysT_pm.astype(bf),
                "values": values_pm.astype(bf),
                "Wq": Wq_pm,
                "Wk": Wk_pm,
                "mask": mask_c.astype(bf),
                "identb": identb,
                "ident": ident,
                "wv32": wv32_pm,
            }
        )
    return (SA, SB), schedule, in_maps


def assemble(schedule, core_outs):
    out = np.zeros((B, Lq, H), dtype=np.float32)
    for core in range(NCORES):
        uA, uB = schedule[core]
        oc = core_outs[core]
        for u, (b, half, _) in ((0, uA), (1, uB)):
            for l in range(64):
                out[b, half * 64 + l, :] = oc[i_phys(u, l), :]
    return out


def kernel(**inputs):
    key, schedule, in_maps = make_in_maps(
        inputs["queries"],
        inputs["keys"],
        inputs["values"],
        inputs["valid_lens"],
        inputs["Wq"],
        inputs["Wk"],
        inputs["wv"],
    )
    nc = _get_program(key)
    res = run_bass_kernel_spmd(nc, in_maps, core_ids=list(range(NCORES)))
    return assemble(schedule, [res.results[c]["out"] for c in range(NCORES)])
